# revision 44
# baseline (speedup 1.0000x reference)
"""MoE ExpertPool kernel for 8 Trainium2 NeuronCores (expert-parallel).

Host side: one expert per core.  Tokens routed to expert e (via either
top-k slot) are gathered and padded to a common capacity C (multiple of
512).  All device tensors are pre-arranged on the host so every DMA is
contiguous per partition and every matmul uses natural [K, M] layouts:

  device (per core):  H = silu(Wg^T @ xT) * (Wu^T @ xT)      [d_expert, C]
                      yT = Wd^T @ H                          [d_model, C]

Activations stay transposed ([feature, token]) the whole way, so the
tokens live on the matmul free dim and weights are the stationary lhsT.
The per-token routing weight and the scatter-add back to (B,S,D) happen
on the host (they are linear post-ops of yT).

Matmuls run as float32r (FP22 multiply, fp32 accumulate): full PE rate
with ~1e-4 relative error.
"""

import numpy as np

D_MODEL = 768
D_EXPERT = 3072
N_EXPERTS = 8
TOP_K = 2
P = 128
KD = D_MODEL // P      # 6   d_model chunks of 128
MD = D_EXPERT // P     # 24  d_expert chunks of 128
TCH = 512              # token chunk = PSUM bank free dim (fp32)
WG_W = 256             # gate/up stationary-weight tile width
N_WG = D_EXPERT // WG_W

_CACHE = {}
LAST_RESULTS = None


def _ensure_axon_hooks():
    """Provide antenv.axon_hooks if the image lacks it, so the trace=True
    path of run_bass_kernel_spmd works (and BASS_TRACE=1 can't crash us)."""
    import sys
    import types

    try:
        import antenv.axon_hooks  # noqa: F401

        return
    except ImportError:
        pass
    try:
        import antenv
    except ImportError:
        return
    mod = types.ModuleType("antenv.axon_hooks")
    mod._hook = None
    mod.set_axon_ntff_profile_hook = lambda h: setattr(mod, "_hook", h)
    mod.get_axon_ntff_profile_hook = lambda: mod._hook
    sys.modules["antenv.axon_hooks"] = mod
    antenv.axon_hooks = mod
    try:
        from trn_agent_boot.trn_boot import _ntff_profile_via_ctypes

        hook = _ntff_profile_via_ctypes("/opt/axon/libaxon_pjrt.so")
        if hook is not None:
            mod._hook = hook
    except Exception:
        pass


def _build(C):
    import concourse.mybir as mybir
    import concourse.tile as tile
    from concourse import bacc

    f32 = mybir.dt.float32
    f32r = mybir.dt.float32r
    Act = mybir.ActivationFunctionType

    nc = bacc.Bacc("TRN2", dynamic_dma_scratch_size=512, num_swdge_queues=4)
    xt = nc.dram_tensor("xt", [P, KD, C], f32r, kind="ExternalInput")
    wg = nc.dram_tensor("wg", [P, N_WG, KD, WG_W], f32r, kind="ExternalInput")
    wu = nc.dram_tensor("wu", [P, N_WG, KD, WG_W], f32r, kind="ExternalInput")
    wd = nc.dram_tensor("wd", [P, KD, MD, P], f32r, kind="ExternalInput")
    yt = nc.dram_tensor("yt", [P, KD, C], f32, kind="ExternalOutput")

    if C % TCH == 0:
        NB, TAIL = C // TCH, 0
    else:
        NB, TAIL = C // TCH, C % TCH
    tail0 = NB * TCH
    # 8 PSUM banks: gps+ups fill them (4+4) when tail-free, else 3+3+2 tail.
    bufs_big = (4 if TAIL == 0 else 3) if NB <= 2 else NB

    with tile.TileContext(nc) as tc:
        with (
            tc.tile_pool(name="singles", bufs=1) as singles,
            tc.tile_pool(name="wpool", bufs=2) as wpool,
            tc.tile_pool(name="tmp", bufs=3) as tmp,
            tc.tile_pool(name="psum", bufs=2, space="PSUM") as psum,
        ):
            xt_sb = singles.tile([P, KD, C], f32r)
            H_sb = singles.tile([P, MD, C], f32r)

            # Prologue: xt k-chunks alternate between the two HWDGE queues
            # (SP + ACT), interleaved with the mo=0 weight chunks, so the
            # first matmul starts right after the framework preamble and the
            # m=0 k-loop is fed at DMA pace without enqueue-rate overhead.
            wg_t0 = wpool.tile([P, KD, WG_W], f32r, tag="wg", name="wg_t0")
            wu_t0 = wpool.tile([P, KD, WG_W], f32r, tag="wu", name="wu_t0")
            # xt / mo=0 weight chunks stream as per-k triples, round-robin
            # across the two HWDGE queues, so data arrives in exactly the
            # order the first two m-groups consume it.
            qrr = [nc.sync, nc.scalar]
            qi = 0
            for k in range(KD):
                for dst, src in (
                    (xt_sb[:, k], xt[:, k]),
                    (wg_t0[:, k], wg[:, 0, k]),
                    (wu_t0[:, k], wu[:, 0, k]),
                ):
                    qrr[qi % 2].dma_start(out=dst, in_=src)
                    qi += 1

            # PE pre-warm: dummy matmuls on a zeroed tile while the first
            # DMAs are in flight, so HAM is at K=8/8 when real matmuls start.
            warm_sb = singles.tile([P, TCH], mybir.dt.bfloat16, name="warm_sb")
            nc.vector.memset(warm_sb[:], 0.0)
            warm_ps = psum.tile([P, TCH], f32, tag="ups", bufs=bufs_big,
                                name="warm_ps")
            for _ in range(11):
                nc.tensor.matmul(
                    warm_ps[:], warm_sb[:, :P], warm_sb[:], start=True, stop=True
                )

            # First two m-groups interleave gate and up per k, paced by the
            # chunk arrivals above; their 8 PSUM groups stay open through the
            # whole xt load so the PE does real work during the DMA window.
            part = []
            if TAIL == 0 and WG_W // P >= 2:
                for mj in range(2):
                    m = mj
                    ms = slice(mj * P, (mj + 1) * P)
                    g_ps = [
                        psum.tile([P, TCH], f32, tag="gps", bufs=bufs_big,
                                  name=f"g_{m}_{b}")
                        for b in range(NB)
                    ]
                    u_ps = [
                        psum.tile([P, TCH], f32, tag="ups", bufs=bufs_big,
                                  name=f"u_{m}_{b}")
                        for b in range(NB)
                    ]
                    part.append((m, ms, g_ps, u_ps))
                for k in range(KD):
                    st, sp = k == 0, k == KD - 1
                    for m, ms, g_ps, u_ps in part:
                        for b in range(NB):
                            nc.tensor.matmul(
                                g_ps[b], wg_t0[:, k, ms],
                                xt_sb[:, k, b * TCH : (b + 1) * TCH],
                                start=st, stop=sp,
                            )
                    for m, ms, g_ps, u_ps in part:
                        for b in range(NB):
                            nc.tensor.matmul(
                                u_ps[b], wu_t0[:, k, ms],
                                xt_sb[:, k, b * TCH : (b + 1) * TCH],
                                start=st, stop=sp,
                            )
                for m, ms, g_ps, u_ps in part:
                    sils = []
                    for b in range(NB):
                        sil = tmp.tile([P, TCH], f32, tag="sil", bufs=2,
                                       name=f"sil_p{m}_{b}")
                        nc.scalar.activation(out=sil[:], in_=g_ps[b], func=Act.Silu)
                        sils.append(sil)
                    for b in range(NB):
                        nc.vector.tensor_mul(
                            H_sb[:, m, b * TCH : (b + 1) * TCH], sils[b], u_ps[b]
                        )

            # gate/up projections + silu*mul -> H   (d_expert = m*128 + p).
            # k-inner over all token chunks: the N=512 streams hide the
            # (slow, fp32) weight loads of the N=TAIL matmuls, which reuse
            # the very same lhsT.
            for mo in range(N_WG):
                if mo == 0:
                    wg_t, wu_t = wg_t0, wu_t0
                else:
                    wg_t = wpool.tile([P, KD, WG_W], f32r, tag="wg")
                    nc.sync.dma_start(out=wg_t[:], in_=wg[:, mo])
                    wu_t = wpool.tile([P, KD, WG_W], f32r, tag="wu")
                    nc.scalar.dma_start(out=wu_t[:], in_=wu[:, mo])
                for mj in range(WG_W // P):
                    m = mo * (WG_W // P) + mj
                    if part and m < 2:
                        continue
                    ms = slice(mj * P, (mj + 1) * P)
                    g_ps = [
                        psum.tile([P, TCH], f32, tag="gps", bufs=bufs_big,
                                  name=f"g_{m}_{b}")
                        for b in range(NB)
                    ]
                    u_ps = [
                        psum.tile([P, TCH], f32, tag="ups", bufs=bufs_big,
                                  name=f"u_{m}_{b}")
                        for b in range(NB)
                    ]
                    t_ps = (
                        psum.tile([P, 2 * TAIL], f32, tag="tailps", bufs=2,
                                  name=f"t_{m}")
                        if TAIL
                        else None
                    )
                    for k in range(KD):
                        st, sp = k == 0, k == KD - 1
                        for b in range(NB):
                            nc.tensor.matmul(
                                g_ps[b],
                                wg_t[:, k, ms],
                                xt_sb[:, k, b * TCH : (b + 1) * TCH],
                                start=st, stop=sp,
                            )
                        if TAIL:
                            nc.tensor.matmul(
                                t_ps[:, :TAIL],
                                wg_t[:, k, ms],
                                xt_sb[:, k, tail0:C],
                                start=st, stop=sp,
                            )
                    sils = []
                    for b in range(NB):
                        sil = tmp.tile([P, TCH], f32, tag="sil", bufs=2,
                                       name=f"sil_{m}_{b}")
                        nc.scalar.activation(out=sil[:], in_=g_ps[b], func=Act.Silu)
                        sils.append(sil)
                    for k in range(KD):
                        st, sp = k == 0, k == KD - 1
                        for b in range(NB):
                            nc.tensor.matmul(
                                u_ps[b],
                                wu_t[:, k, ms],
                                xt_sb[:, k, b * TCH : (b + 1) * TCH],
                                start=st, stop=sp,
                            )
                        if TAIL:
                            nc.tensor.matmul(
                                t_ps[:, TAIL:],
                                wu_t[:, k, ms],
                                xt_sb[:, k, tail0:C],
                                start=st, stop=sp,
                            )
                    for b in range(NB):
                        nc.vector.tensor_mul(
                            H_sb[:, m, b * TCH : (b + 1) * TCH], sils[b], u_ps[b]
                        )
                    if TAIL:
                        silt = tmp.tile([P, TAIL], f32, tag="silt", bufs=2,
                                        name=f"silt_{m}")
                        nc.scalar.activation(
                            out=silt[:], in_=t_ps[:, :TAIL], func=Act.Silu
                        )
                        nc.vector.tensor_mul(
                            H_sb[:, m, tail0:C], silt[:], t_ps[:, TAIL:]
                        )

            # down projection   (d_model = n*128 + p); reuses the gps/tailps
            # PSUM tags so the kernel stays within 8 banks.
            MDH = MD // 2
            for n in range(KD):
                wd_h = []
                for h in range(2):
                    wd_t = wpool.tile(
                        [P, MDH, P], f32r, tag="wd", bufs=3, name=f"wd_{n}_{h}"
                    )
                    eng = nc.sync if h == 0 else nc.scalar
                    eng.dma_start(
                        out=wd_t[:], in_=wd[:, n, h * MDH : (h + 1) * MDH]
                    )
                    wd_h.append(wd_t)
                y_ps = [
                    psum.tile([P, TCH], f32, tag="gps", bufs=bufs_big,
                              name=f"y_{n}_{b}")
                    for b in range(NB)
                ]
                yt_ps = (
                    psum.tile([P, TAIL], f32, tag="tailps", bufs=2,
                              name=f"yt_{n}")
                    if TAIL
                    else None
                )
                for k in range(MD):
                    st, sp = k == 0, k == MD - 1
                    lhs = wd_h[k // MDH][:, k % MDH, :]
                    for b in range(NB):
                        nc.tensor.matmul(
                            y_ps[b],
                            lhs,
                            H_sb[:, k, b * TCH : (b + 1) * TCH],
                            start=st, stop=sp,
                        )
                    if TAIL:
                        nc.tensor.matmul(
                            yt_ps, lhs, H_sb[:, k, tail0:C], start=st, stop=sp
                        )
                for b in range(NB):
                    y_sb = tmp.tile([P, TCH], f32, tag="ysb", bufs=2,
                                    name=f"ysb_{n}_{b}")
                    nc.any.tensor_copy(out=y_sb[:], in_=y_ps[b])
                    (nc.sync if b % 2 == 0 else nc.scalar).dma_start(
                        out=yt[:, n, b * TCH : (b + 1) * TCH], in_=y_sb[:]
                    )
                if TAIL:
                    yt_sb = tmp.tile([P, TAIL], f32, tag="ytsb", bufs=2,
                                     name=f"ytsb_{n}")
                    nc.any.tensor_copy(out=yt_sb[:], in_=yt_ps[:])
                    nc.sync.dma_start(out=yt[:, n, tail0:C], in_=yt_sb[:])
    nc.finalize()
    return nc


def kernel(**inputs):
    global LAST_RESULTS
    x = np.ascontiguousarray(np.asarray(inputs["x"], dtype=np.float32))
    rw = np.asarray(inputs["routing_weights"], dtype=np.float32)
    ei = np.asarray(inputs["expert_indices"])
    wg = np.asarray(inputs["w_gate"], dtype=np.float32)
    wu = np.asarray(inputs["w_up"], dtype=np.float32)
    wd = np.asarray(inputs["w_down"], dtype=np.float32)

    B, S, D = x.shape
    T = B * S
    xf = x.reshape(T, D)
    eif = ei.reshape(T, TOP_K).astype(np.int64)
    rwf = rw.reshape(T, TOP_K)

    # per-token weight for each expert (sum over top-k slots assigned to e)
    tokw = np.zeros((T, N_EXPERTS), np.float32)
    np.add.at(tokw, (np.arange(T)[:, None], eif), rwf)

    idxs = [np.nonzero((eif == e).any(axis=1))[0] for e in range(N_EXPERTS)]
    maxc = max(len(i) for i in idxs)
    # Capacity: multiple of 512 so every matmul is a full-width N=512 stream
    # (tail-free kernels are stream-bound, not LDW-bound), covering >=90% of
    # the busiest expert, capped at 1024 so xt+H stay within SBUF.  The few
    # overflow tokens are computed exactly on the host.
    C = TCH * max(1, min(2, -(-(maxc * 9 // 10) // TCH)))

    _ensure_axon_hooks()
    from concourse.bass_utils import run_bass_kernel_spmd

    nc = _CACHE.get(C)
    if nc is None:
        nc = _CACHE[C] = _build(C)

    in_maps = []
    for e in range(N_EXPERTS):
        idx = idxs[e][:C]
        xe = np.zeros((C, D), np.float32)
        xe[: len(idx)] = xf[idx]
        in_maps.append(
            {
                "xt": np.ascontiguousarray(xe.T.reshape(KD, P, C).transpose(1, 0, 2)),
                "wg": np.ascontiguousarray(
                    wg[e].reshape(KD, P, N_WG, WG_W).transpose(1, 2, 0, 3)
                ),
                "wu": np.ascontiguousarray(
                    wu[e].reshape(KD, P, N_WG, WG_W).transpose(1, 2, 0, 3)
                ),
                "wd": np.ascontiguousarray(
                    wd[e].reshape(MD, P, KD, P).transpose(1, 2, 0, 3)
                ),
            }
        )

    try:
        res = run_bass_kernel_spmd(nc, in_maps, core_ids=list(range(N_EXPERTS)))
    except Exception:
        # transient NRT/device hiccups (e.g. NRT_EXEC_UNIT_UNRECOVERABLE)
        # usually clear on a retry
        res = run_bass_kernel_spmd(nc, in_maps, core_ids=list(range(N_EXPERTS)))
    LAST_RESULTS = res

    out = np.zeros((T, D), np.float32)
    for e in range(N_EXPERTS):
        idx = idxs[e][:C]
        ye = res.results[e]["yt"].transpose(1, 0, 2).reshape(D, C).T
        out[idx] += ye[: len(idx)] * tokw[idx, e][:, None]
        spill = idxs[e][C:]
        if len(spill):
            xs = xf[spill]
            h = xs @ wg[e]
            h = (h / (1.0 + np.exp(-h))) * (xs @ wu[e])
            out[spill] += (h @ wd[e]) * tokw[spill, e][:, None]
    return out.reshape(B, S, D)
